# revision 31
# baseline (speedup 1.0000x reference)
"""Causal self-attention (RoPE + qk-RMS-norm) Trainium2 kernel.

Sharding: 8 cores = 2 batches x 4 head-groups (tensor-parallel over heads,
data-parallel over batch). Each core computes its head-group's attention and
a row-parallel partial of the output projection; the host sums the 4
per-group partials per batch (the all-reduce of row-parallel sharding).

Per-core pipeline (single pass over T, fp16 operands, f32 PSUM):
- QK projection, rope and rms-norm are fused per (q|k, 512-token supertile)
  unit with one-unit software pipelining so the PE never waits on the
  vector/ACT chains. Rope runs entirely on the vector engine via
  partition-crossing adds (the stacked [cos;sin] / [sin;cos] layout makes
  each output half depend on a single elementwise product).
- Attention is transposed-flash: S.T = K @ Q.T per 128-token key block so
  P.T feeds the PV matmul directly; no max-subtraction (scores of
  rms-normed q,k are bounded); exp carries a -1 bias that cancels in the
  normalizer. The softmax denominator accumulates on the vector engine
  (P.T block adds) with a single ones-matmul reduction per supertile,
  keeping the PE for real MACs. 1/sum is deferred to Y.T.
- V projection is emitted after head 0's fused block (x DMA cover); the
  output projection is split around the last head's group-B attention.
"""

import functools

import numpy as np

B, T, C, H, D = 2, 2048, 1280, 10, 128
EPS = 1e-5
NHL = 3  # head slots per core (padded)
N_CORES = 8
CCH = C // 128  # contraction chunks
TBN = T // 128  # 128-token blocks
Q4 = T // 512  # 512-query supertiles
NG = 2  # attention query-supertile groups per head
Q42 = Q4 // NG
HD = NHL * D
EXPB = -1.0  # exp bias; cancels in the normalizer
# per-batch head groups (4th group padded with zero heads)
GROUPS = [[0, 1, 2], [3, 4, 5], [6, 7, 8], [9]]
COUTS = [(0, 512), (512, 512), (1024, 256)]


def _emit(nc, tile, mybir):
    F32 = mybir.dt.float32
    F16 = mybir.dt.float16
    ActF = mybir.ActivationFunctionType

    xt = nc.dram_tensor("xt", [128, CCH, T], F16, kind="ExternalInput")
    wqt = nc.dram_tensor("wqt", [128, CCH, HD], F16, kind="ExternalInput")
    wkt = nc.dram_tensor("wkt", [128, CCH, HD], F16, kind="ExternalInput")
    wvt = nc.dram_tensor("wvt", [128, CCH, HD], F16, kind="ExternalInput")
    wpt = nc.dram_tensor("wpt", [HD, C], F16, kind="ExternalInput")
    cs = nc.dram_tensor("cs", [D, T], F16, kind="ExternalInput")
    sc = nc.dram_tensor("sc", [D, T], F16, kind="ExternalInput")
    out = nc.dram_tensor("out", [T, C], F16, kind="ExternalOutput")
    warm = nc.dram_tensor("warm", [1, 512], F32, kind="ExternalOutput")

    from contextlib import ExitStack

    with ExitStack() as ctx:
        ctx.enter_context(nc.allow_low_precision(reason="fp16 matmul operands"))
        tc = ctx.enter_context(tile.TileContext(nc))
        pool = lambda n, b, **kw: ctx.enter_context(tc.tile_pool(name=n, bufs=b, **kw))
        drp = pool("dr", 2, space="DRAM")
        per = pool("persist", 1)
        wts = pool("wts", 1)
        xcp = pool("xc", 1)
        qtp = pool("qt", 3)
        ytp = pool("yt", 1)
        tmp = pool("tmp", 2)
        sqp = pool("sqp", 2)
        ptp = pool("ptp", 3)
        csa = pool("csa", 2)
        rows = pool("rows", 2)
        oev = pool("oev", 4)
        ps1 = pool("ps1", 4, space="PSUM")
        psacc = pool("psacc", 2, space="PSUM")
        psrow = pool("psrow", 2, space="PSUM")

        # ---- tiny constants the warmup needs, on gpsimd before its DMAs ----
        ones_col = per.tile([128, 1], F16, tag="onc")
        nc.gpsimd.memset(ones_col[:], 1.0)
        wrhs = per.tile([128, 512], F16, tag="wrhs")
        nc.gpsimd.memset(wrhs[:], 1.0)

        # ---- input DMAs, round-robin across the three DMA-capable queues ----
        wq = wts.tile([128, CCH, HD], F16, tag="wq")
        wk = wts.tile([128, CCH, HD], F16, tag="wk")
        wv = wts.tile([128, CCH, HD], F16, tag="wv")
        nc.sync.dma_start(wq[:], wqt[:])
        nc.scalar.dma_start(wk[:], wkt[:])
        xc = []
        for c in range(CCH):
            t = xcp.tile([128, T], F16, tag=f"x{c}")
            eng = (nc.gpsimd, nc.sync, nc.scalar)[c % 3]
            eng.dma_start(t[:], xt[:, c, :])
            xc.append(t)
        nc.scalar.dma_start(wv[:], wvt[:])  # first needed by Vproj, much later
        cs_t = per.tile([D, T], F16, tag="cs")
        sc_t = per.tile([D, T], F16, tag="sc")
        nc.sync.dma_start(cs_t[:], cs[:])
        nc.sync.dma_start(sc_t[:], sc[:])
        wp = {}
        for hh in range(NHL):
            for ci, (co, cw) in enumerate(COUTS):
                t = wts.tile([128, cw], F16, tag=f"wp{hh}_{ci}")
                nc.scalar.dma_start(t[:], wpt[hh * 128 : (hh + 1) * 128, co : co + cw])
                wp[(hh, ci)] = t

        # ---- remaining constants ----
        ones_row = per.tile([1, 128], F16, tag="onr")
        nc.vector.memset(ones_row[:], 1.0)
        expb_col = per.tile([128, 1], F32, tag="expb")
        nc.vector.memset(expb_col[:], float(EXPB))
        beps_row = per.tile([1, 1], F32, tag="beps")
        nc.vector.memset(beps_row[:], float(EPS))
        zero_row = per.tile([1, 1], F32, tag="zrow")
        nc.vector.memset(zero_row[:], 0.0)
        # rope half-mix selectors, M padded to 128 with disjoint columns:
        # y[0:64] = MA.T@t1 (cols 64-127 zero), y[64:128] = MB.T@t2
        # (the two matmuls accumulate in PSUM).
        ma_f = oev.tile([128, 128], F32, tag="ot", name="ma_f")
        mb_f = oev.tile([128, 128], F32, tag="ot", name="mb_f")
        nc.gpsimd.memset(ma_f[:], 0.0)
        nc.gpsimd.memset(mb_f[:], 0.0)
        nc.gpsimd.affine_select(
            out=ma_f[:, 0:64], in_=ma_f[:, 0:64],
            compare_op=mybir.AluOpType.not_equal,
            fill=1.0, base=0, pattern=[[-1, 64]], channel_multiplier=1,
        )
        nc.gpsimd.affine_select(
            out=ma_f[:, 0:64], in_=ma_f[:, 0:64],
            compare_op=mybir.AluOpType.not_equal,
            fill=1.0, base=-64, pattern=[[-1, 64]], channel_multiplier=1,
        )
        nc.gpsimd.affine_select(
            out=mb_f[:, 64:128], in_=mb_f[:, 64:128],
            compare_op=mybir.AluOpType.not_equal,
            fill=-1.0, base=0, pattern=[[-1, 64]], channel_multiplier=1,
        )
        nc.gpsimd.affine_select(
            out=mb_f[:, 64:128], in_=mb_f[:, 64:128],
            compare_op=mybir.AluOpType.not_equal,
            fill=1.0, base=-64, pattern=[[-1, 64]], channel_multiplier=1,
        )
        ma = per.tile([128, 128], F16, tag="ma")
        mb = per.tile([128, 128], F16, tag="mb")
        nc.scalar.copy(ma[:], ma_f[:])
        nc.scalar.copy(mb[:], mb_f[:])

        # PE warm-up: dummy accumulating matmuls during the initial DMA ramp
        wps = psrow.tile([1, 512], F32, tag="row", name="warmps")
        NWARM = 24
        for i in range(NWARM):
            nc.tensor.matmul(
                wps[:], ones_col[:], wrhs[:], start=(i == 0), stop=(i == NWARM - 1)
            )
        wsb = rows.tile([1, 512], F32, tag="rw", name="warmsb")
        nc.vector.tensor_copy(wsb[:], wps[:])
        nc.sync.dma_start(warm[:], wsb[:])

        # V for all heads/all tokens: [tk-part, h, tb, d] fp16
        v_t = per.tile([128, NHL, TBN, D], F16, tag="v")
        # K.T per head, all tokens (fp16, rope'd, un-normalized)
        ktr = [per.tile([128, T], F16, tag=f"ktr{h}", name=f"ktr{h}")
               for h in range(NHL)]
        rk_cols = [per.tile([128, TBN], F32, tag=f"rkc{h}", name=f"rkc{h}")
                   for h in range(NHL)]
        # Y.T all heads (fp16)
        ytn = ytp.tile([128, NHL, T], F16, tag="ytn")

        def emit_fused(h, qtn):
            """QK projection + rope + norm for head h, one-unit software
            pipeline over interleaved (k,q) supertile units so the PE stays
            dense while the vector/ACT chains of the previous unit complete.
            The q-norm row ops are batched once per head; their chain hides
            under the previous head's attention."""
            rkrow = rows.tile([1, T], F32, tag="rkrow", bufs=1)

            def unit_tail(isq, q4, qp):
                gsl = slice(q4 * 512, (q4 + 1) * 512)
                dst = ktr[h] if isq else qtn
                t1 = tmp.tile([128, 512], F16, tag="t1")
                t2 = tmp.tile([128, 512], F16, tag="t2")
                nc.vector.tensor_mul(t1[:], qp[:], cs_t[:, gsl])
                nc.vector.tensor_mul(t2[:], qp[:], sc_t[:, gsl])
                rp = ps1.tile([128, 512], F32, tag="mm", name="rp")
                nc.tensor.matmul(rp[:], ma[:], t1[:], start=True, stop=False)
                nc.tensor.matmul(rp[:], mb[:], t2[:], start=False, stop=True)
                nc.scalar.copy(dst[:, gsl], rp[:])
                sq = sqp.tile([128, 512], F16, tag="sq")
                nc.vector.tensor_mul(sq[:], rp[:], dst[:, gsl])
                ss = psrow.tile([1, 512], F32, tag="row", name="ss")
                nc.tensor.matmul(ss[:], ones_col[:], sq[:], start=True, stop=True)
                if isq:
                    # rk row: sqrt(ssk/D + eps); recip after the transpose
                    nc.scalar.activation(
                        rkrow[:, gsl], ss[:], ActF.Sqrt,
                        scale=1.0 / D, bias=beps_row[0:1, :],
                    )
                else:
                    # q: rq = sqrt(1/ssq) (folds 1/sqrt(D); no eps -- pad
                    # heads get nonzero Wq host-side), applied to qtn
                    # columns via ones-outer broadcast; the chain hides
                    # under the next interleaved unit's projection
                    rw = rows.tile([1, 512], F32, tag="rw")
                    nc.vector.reciprocal_approx_fast(rw[:], ss[:])
                    rwr = rows.tile([1, 512], F16, tag="rwr", bufs=2)
                    nc.scalar.activation(
                        rwr[:], rw[:], ActF.Sqrt, bias=zero_row[0:1, :]
                    )
                    bq = ps1.tile([128, 512], F32, tag="mm", name="bq")
                    nc.tensor.matmul(
                        bq[:], ones_row[:], rwr[:], start=True, stop=True
                    )
                    nc.vector.tensor_mul(qtn[:, gsl], qtn[:, gsl], bq[:])

            units = [(p, q4) for q4 in range(Q4) for p in (1, 0)]
            prev = None
            for isq, q4 in units:
                qp = ps1.tile([128, 512], F32, tag="mm", name="qp")
                wt = wk if isq else wq
                for c in range(CCH):
                    nc.tensor.matmul(
                        qp[:],
                        wt[:, c, h * D : (h + 1) * D],
                        xc[c][:, q4 * 512 : (q4 + 1) * 512],
                        start=(c == 0), stop=(c == CCH - 1),
                    )
                if prev is not None:
                    unit_tail(*prev)
                    if prev[:2] == (1, Q4 - 1):
                        # k side done: transpose rk [1,T] -> [128,TBN] via a
                        # DRAM bounce, then the cheap 128-lane reciprocal
                        rkd = drp.tile([1, T], F32, tag="rkd")
                        nc.sync.dma_start(rkd[:], rkrow[:])
                        rksq = rows.tile([128, TBN], F32, tag="rksq", bufs=1)
                        nc.sync.dma_start(
                            rksq[:],
                            rkd[0:1, :].rearrange("a (j p) -> a p j", p=128),
                        )
                        nc.vector.reciprocal_approx_fast(rk_cols[h][:], rksq[:])
                prev = (isq, q4, qp)
            unit_tail(*prev)

        def emit_vproj(inject=None):
            for tb in range(TBN):
                vp = ps1.tile([128, HD], F32, tag="mm", name="vp")
                for c in range(CCH):
                    nc.tensor.matmul(
                        vp[:],
                        xc[c][:, tb * 128 : (tb + 1) * 128],
                        wv[:, c, :],
                        start=(c == 0), stop=(c == CCH - 1),
                    )
                nc.vector.tensor_copy(v_t[:, :, tb, :], vp[:])
                if tb == 2 and inject is not None:
                    inject()
                    inject = None

        def emit_outproj(tbs):
            for n, tb in enumerate(tbs):
                for ci, (co, cw) in enumerate(COUTS):
                    k = n * len(COUTS) + ci
                    p = (psacc, ps1)[k % 2]
                    op = p.tile([128, cw], F32,
                                tag="acc" if p is psacc else "mm", name="op")
                    for hh in range(NHL):
                        nc.tensor.matmul(
                            op[:],
                            ytn[:, hh, tb * 128 : (tb + 1) * 128],
                            wp[(hh, ci)][:],
                            start=(hh == 0), stop=(hh == NHL - 1),
                        )
                    ot = oev.tile([128, cw], F16, tag="ot")
                    if k % 2 == 0:
                        nc.vector.tensor_copy(ot[:], op[:])
                    else:
                        nc.scalar.copy(ot[:], op[:])
                    (nc.sync, nc.scalar)[k % 2].dma_start(
                        out[tb * 128 : (tb + 1) * 128, co : co + cw], ot[:]
                    )

        def emit_attn_group(h, qtn, g, inject=None):
            """One query-supertile group of head h's attention, kb-pipelined
            (st/exp run LA kb ahead of PV). The softmax denominator
            accumulates on the vector engine; one ones-matmul per supertile
            reduces it at the end. Returns the normalizer closure."""
            gq4s = [g * Q42 + i for i in range(Q42)]
            goff = g * Q42 * 512
            yts = [psacc.tile([128, 512], F32, tag="acc", name=f"yt{i}")
                   for i in range(Q42)]
            accs = [None] * Q42
            kbmax = 4 * (gq4s[-1] + 1)
            LA = 2
            pts = {}
            for kb in range(kbmax + LA):
                if kb < kbmax:
                    active = [i for i in range(Q42) if kb <= 4 * gq4s[i] + 3]
                    i0 = active[0]
                    pt = ptp.tile([128, Q42 * 512], F16, tag="pt")
                    j0 = kb - 4 * gq4s[i0]
                    for i in active:
                        st = ps1.tile([128, 512], F32, tag="mm", name="st")
                        nc.tensor.matmul(
                            st[:],
                            ktr[h][:, kb * 128 : (kb + 1) * 128],
                            qtn[:, goff + i * 512 : goff + (i + 1) * 512],
                            start=True, stop=True,
                        )
                        lo = (j0 * 128 if (i == i0 and j0 > 0) else 0)
                        nc.scalar.activation(
                            pt[:, i * 512 + lo : (i + 1) * 512],
                            st[:, lo:512], ActF.Exp,
                            scale=rk_cols[h][:, kb : kb + 1], bias=expb_col[:],
                        )
                    if j0 > 0:
                        nc.gpsimd.memset(pt[:, i0 * 512 : i0 * 512 + j0 * 128], 0.0)
                    if 0 <= j0 <= 3:
                        dg = slice(i0 * 512 + j0 * 128, i0 * 512 + (j0 + 1) * 128)
                        # keep tq >= tk in [tk, tq] layout
                        nc.gpsimd.affine_select(
                            out=pt[:, dg], in_=pt[:, dg],
                            compare_op=mybir.AluOpType.is_ge,
                            fill=0.0, base=0,
                            pattern=[[1, 128]], channel_multiplier=-1,
                        )
                    # denominator accumulation on the vector engine
                    for i in active:
                        lsl = slice(i * 512, (i + 1) * 512)
                        if accs[i] is None:
                            accs[i] = csa.tile(
                                [128, 512], F16, tag=f"acc{i}", name=f"acc{i}"
                            )
                            nc.vector.tensor_copy(accs[i][:], pt[:, lsl])
                        else:
                            nc.vector.tensor_add(
                                accs[i][:], accs[i][:], pt[:, lsl]
                            )
                    pts[kb] = pt
                if kb == 2 and inject is not None:
                    inject()
                    inject = None
                if kb >= LA:
                    pkb = kb - LA
                    pt = pts.pop(pkb)
                    for i in range(Q42):
                        lastkb = 4 * gq4s[i] + 3
                        if pkb > lastkb:
                            continue
                        nc.tensor.matmul(
                            yts[i][:], v_t[:, h, pkb, :],
                            pt[:, i * 512 : (i + 1) * 512],
                            start=(pkb == 0), stop=(pkb == lastkb),
                        )
            csrs = []
            for i in range(Q42):
                csf = psrow.tile([1, 512], F32, tag="row", name="csf")
                nc.tensor.matmul(
                    csf[:], ones_col[:], accs[i][:], start=True, stop=True
                )
                csr = rows.tile([1, 512], F16, tag="rw", name="csr")
                nc.vector.tensor_copy(csr[:], csf[:])
                csrs.append(csr)

            def normalize(goff=goff, yts=yts, csrs=csrs):
                for i in range(Q42):
                    gsl = slice(goff + i * 512, goff + (i + 1) * 512)
                    bc = ps1.tile([128, 512], F32, tag="mm", name="bc")
                    nc.tensor.matmul(
                        bc[:], ones_row[:], csrs[i][:], start=True, stop=True
                    )
                    bcs = tmp.tile([128, 512], F32, tag="t1", name="bcs")
                    nc.vector.reciprocal_approx_fast(bcs[:], bc[:])
                    nc.vector.tensor_mul(ytn[:, h, gsl], yts[i][:], bcs[:])

            return normalize

        def emit_attention(h, qtn, last=False, inject=None):
            nrm_a = emit_attn_group(h, qtn, 0, inject=inject)
            if last:
                nrm_a()
                emit_outproj(range(0, TBN // 2))
                nrm_b = emit_attn_group(h, qtn, 1)
                nrm_b()
                emit_outproj(range(TBN // 2, TBN))
                return None
            nrm_a()
            return emit_attn_group(h, qtn, 1)

        # ---- schedule ----
        qtn0 = qtp.tile([128, T], F16, tag="qtn")
        emit_fused(0, qtn0)
        emit_vproj()  # dense PE; covers the tail of h0's chains

        pending = lambda: emit_attention(0, qtn0)
        for h in range(1, NHL):
            qtn = qtp.tile([128, T], F16, tag="qtn")
            emit_fused(h, qtn)
            norm_prev = pending()
            if norm_prev is not None:
                norm_prev()
            pending = (lambda h=h, qtn=qtn, last=(h == NHL - 1):
                       emit_attention(h, qtn, last))
        pending()
    return nc


@functools.lru_cache(maxsize=4)
def _build():
    import concourse.bacc as bacc
    import concourse.tile as tile
    from concourse import mybir

    nc = bacc.Bacc("TRN2", target_bir_lowering=False)
    _emit(nc, tile, mybir)
    nc.compile()
    return nc


def _pack_chunks(a):
    """[C, N] -> [128, CCH, N]: partition-major chunk layout."""
    return np.ascontiguousarray(a.reshape(CCH, 128, -1).transpose(1, 0, 2))


def _shard(x, cos, sin, Wq, Wk, Wv, Wproj):
    """Build the 8 per-core input maps."""
    F16 = np.float16
    cosT = np.ascontiguousarray(cos[0, 0].T.astype(np.float32))  # [64, T]
    sinT = np.ascontiguousarray(sin[0, 0].T.astype(np.float32))
    cs = np.concatenate([cosT, sinT], axis=0).astype(F16)  # [128, T]
    sc = np.concatenate([sinT, cosT], axis=0).astype(F16)

    def head_rows(W, heads, pad=0.0):
        rows = np.full((HD, C), pad, np.float32)
        for i, h in enumerate(heads):
            rows[i * D : (i + 1) * D] = W[h * D : (h + 1) * D]
        return rows

    in_maps = []
    for b in range(B):
        xtb = _pack_chunks(x[b].T.astype(np.float32)).astype(F16)
        for heads in GROUPS:
            wqp = _pack_chunks(head_rows(Wq, heads, pad=0.01).T).astype(F16)
            wkp = _pack_chunks(head_rows(Wk, heads).T).astype(F16)
            wvp = _pack_chunks(head_rows(Wv, heads).T).astype(F16)
            # Wproj columns for these heads, transposed: [HD, C]
            wpp = np.zeros((HD, C), np.float32)
            for i, h in enumerate(heads):
                wpp[i * D : (i + 1) * D] = Wproj[:, h * D : (h + 1) * D].T
            in_maps.append(
                {"xt": xtb, "wqt": wqp, "wkt": wkp, "wvt": wvp,
                 "wpt": wpp.astype(F16), "cs": cs, "sc": sc}
            )
    return in_maps


def _gather(results):
    y = np.zeros((B, T, C), np.float32)
    for b in range(B):
        for g in range(len(GROUPS)):
            y[b] += results[b * len(GROUPS) + g]["out"].astype(np.float32)
    return y


def _run(in_maps, trace=False):
    from concourse.bass_utils import run_bass_kernel_spmd

    nc = _build()
    return run_bass_kernel_spmd(
        nc, in_maps, core_ids=list(range(N_CORES)), trace=trace
    )


def kernel(x, cos, sin, Wq, Wk, Wv, Wproj):
    ins = _shard(
        np.asarray(x), np.asarray(cos), np.asarray(sin),
        np.asarray(Wq), np.asarray(Wk), np.asarray(Wv), np.asarray(Wproj),
    )
    res = _run(ins, trace=False)
    return _gather(res.results)


def run_traced(x, cos, sin, Wq, Wk, Wv, Wproj):
    ins = _shard(
        np.asarray(x), np.asarray(cos), np.asarray(sin),
        np.asarray(Wq), np.asarray(Wk), np.asarray(Wv), np.asarray(Wproj),
    )
    res = _run(ins, trace=True)
    return _gather(res.results), res
